# revision 24
# baseline (speedup 1.0000x reference)
"""DimensionWiseMINE on 8 Trainium2 NeuronCores.

Sharding: stage-1 gene projector x@Wg1 is contraction-sharded (XD/8 per core)
with a chunked bf16 AllReduce of the pre-activation overlapping the matmul;
everything downstream of h is expert-parallel over the D=64 per-dim nets
(8 dims per core). The batch permutation is applied on the host to z (the
batch sum is permutation invariant), so no on-device gather is needed.

Layouts are feature-major ("transposed": [feature_partition, batch_free])
throughout, so no on-device transposes are needed.

ELU is computed as  elu(y)+1 = max(y+1, min(exp(y), 1))  -- one ACT op
(exp with bias -1 reading y+1) plus one fused DVE scalar_tensor_tensor op.
The +1 shifts are absorbed into the next layer's bias via weight row/col
sums, and all biases ride the matmuls as extra contraction rows (hi/lo bf16
split for accuracy) or ACT bias slots.
"""

import numpy as np
import ml_dtypes

import concourse.bass as bass
import concourse.bacc as bacc
import concourse.tile as tile
from concourse import mybir
from concourse.bass_utils import run_bass_kernel_spmd

NCORES = 8
B, XD, PD, D, H, G1 = 512, 32768, 512, 64, 128, 1024
KSH = XD // NCORES   # 4096 contraction slice per core
DL = D // NCORES     # 8 local dims per core
NKT1 = KSH // 128    # 32 k-tiles for MM1
NG = 4               # MM1 m-groups (each 2 m-tiles of g1) / AllReduce chunks
F32 = mybir.dt.float32
BF16 = mybir.dt.bfloat16
BF = ml_dtypes.bfloat16
AF = mybir.ActivationFunctionType
OP = mybir.AluOpType

_cache = {}


def _build(sim=False, stage=3):
    nc = bacc.Bacc("TRN2", target_bir_lowering=False, debug=False,
                   num_devices=1 if sim else NCORES)

    xt_d = nc.declare_dram_parameter("xt", [8, 128, (NKT1 // 8) * B], BF16, isOutput=False)
    wg1_d = nc.declare_dram_parameter("wg1", [NG, 4, 128, (NKT1 // 4) * 256], BF16, isOutput=False)
    bg1c_d = nc.declare_dram_parameter("bg1c", [128, 8], F32, isOutput=False)
    wg2_d = nc.declare_dram_parameter("wg2", [128, 8 * PD], BF16, isOutput=False)
    b2r_d = nc.declare_dram_parameter("b2r", [2, PD], BF16, isOutput=False)
    wh_d = nc.declare_dram_parameter("wh", [128, 4 * DL * H], BF16, isOutput=False)
    l1b_d = nc.declare_dram_parameter("l1b", [3, DL * H], BF16, isOutput=False)
    zj_d = nc.declare_dram_parameter("zj", [3, DL * B], BF16, isOutput=False)
    zm_d = nc.declare_dram_parameter("zm", [3, DL * B], BF16, isOutput=False)
    w2_d = nc.declare_dram_parameter("w2", [128, DL * H], BF16, isOutput=False)
    l2b_d = nc.declare_dram_parameter("l2b", [2, DL * H], BF16, isOutput=False)
    ones_d = nc.declare_dram_parameter("ones2", [2, B], BF16, isOutput=False)
    w3_d = nc.declare_dram_parameter("w3", [128, DL], BF16, isOutput=False)
    cvec_d = nc.declare_dram_parameter("cvec", [1, DL], F32, isOutput=False)
    out_d = nc.declare_dram_parameter("out", [144], F32, isOutput=True)

    arin = nc.dram_tensor("arin", [8, 128, B], BF16)
    arout = nc.dram_tensor("arout", [8, 128, B], BF16, addr_space="Shared")

    with tile.TileContext(nc) as tc:
        with (
            tc.tile_pool(name="wg1p", bufs=2) as wg1p,
            tc.tile_pool(name="castp", bufs=4) as castp,
            tc.tile_pool(name="consts", bufs=1) as cst,
            tc.tile_pool(name="work", bufs=1) as wk,
            tc.tile_pool(name="ep", bufs=4) as ep,
            tc.tile_pool(name="ap_", bufs=6) as app,
            tc.tile_pool(name="mep", bufs=2) as mep,
        ):
            # ---- startup DMAs in priority order: first MM1 operands,
            # then the rest of x / Wg1 group 0, then constants ----
            XC = (NKT1 // 8) * B
            xtt = [cst.tile([128, XC], BF16, name=f"xts_{i}") for i in range(8)]
            WQ = (NKT1 // 4) * 256
            wg1h = [[wg1p.tile([128, WQ], BF16, tag=f"wg1h{h}",
                               name=f"wg_{g}_{h}", bufs=3) for h in range(4)]
                    for g in range(NG)]
            # interleave so the first matmuls' operands land first
            nc.sync.dma_start(xtt[0][:], xt_d[0])
            nc.sync.dma_start(wg1h[0][0][:], wg1_d[0, 0])
            nc.sync.dma_start(xtt[1][:], xt_d[1])
            nc.sync.dma_start(wg1h[0][1][:], wg1_d[0, 1])
            nc.sync.dma_start(xtt[2][:], xt_d[2])
            nc.sync.dma_start(wg1h[0][2][:], wg1_d[0, 2])
            nc.sync.dma_start(xtt[3][:], xt_d[3])
            nc.sync.dma_start(wg1h[0][3][:], wg1_d[0, 3])
            for i in range(4, 8):
                nc.sync.dma_start(xtt[i][:], xt_d[i])
            for h in range(4):
                nc.sync.dma_start(wg1h[1][h][:], wg1_d[1, h])
            # ---- constants in (lower priority) ----
            bg1c = cst.tile([128, 8], F32)
            nc.scalar.dma_start(bg1c[:], bg1c_d[:])
            wg2sb = cst.tile([128, 8 * PD], BF16)
            nc.scalar.dma_start(wg2sb[:], wg2_d[:])
            b2r = cst.tile([2, PD], BF16)
            nc.scalar.dma_start(b2r[:], b2r_d[:])
            wht = cst.tile([128, 4 * DL * H], BF16)
            nc.scalar.dma_start(wht[:], wh_d[:])
            l1b = cst.tile([3, DL * H], BF16)
            nc.scalar.dma_start(l1b[:], l1b_d[:])
            zj = cst.tile([3, DL * B], BF16)
            nc.scalar.dma_start(zj[:], zj_d[:])
            zm = cst.tile([3, DL * B], BF16)
            nc.scalar.dma_start(zm[:], zm_d[:])
            w2t = cst.tile([128, DL * H], BF16)
            nc.scalar.dma_start(w2t[:], w2_d[:])
            l2b = cst.tile([2, DL * H], BF16)
            nc.scalar.dma_start(l2b[:], l2b_d[:])
            ones2 = cst.tile([2, B], BF16)
            nc.scalar.dma_start(ones2[:], ones_d[:])
            w3t = cst.tile([128, DL], BF16)
            nc.scalar.dma_start(w3t[:], w3_d[:])
            cvec = cst.tile([1, DL], F32)
            nc.scalar.dma_start(cvec[:], cvec_d[:])

            # warm the exp table set early so the ~2.7us load hides under MM1
            wz0 = cst.tile([128, 1], F32)
            nc.vector.memset(wz0[:], 0.0)
            wz1 = cst.tile([128, 1], F32)
            nc.scalar.activation(wz1[:], wz0[:], AF.Exp)
            neg1 = cst.tile([128, 1], F32)
            nc.vector.memset(neg1[:], -1.0)


            h1m = [wk.tile([128, B], BF16, name=f"h1m_{m}") for m in range(8)]
            e1m = [wk.tile([128, B], BF16, name=f"e1m_{m}") for m in range(8)]
            a1m = [wk.tile([128, B], BF16, name=f"a1m_{m}") for m in range(8)]
            htm = [wk.tile([128, B], BF16, name=f"htm_{mt}") for mt in range(4)]

            with (
                tc.tile_pool(name="ps1", bufs=4, space="PSUM") as ps1,
                tc.tile_pool(name="ps2p", bufs=4, space="PSUM") as ps2p,
            ):
                ps2 = [ps2p.tile([128, B], F32, tag="psmm2", name=f"psmm2_{i}")
                       for i in range(4)]
                # ---- MM1 k-contiguous per m-group + chunked AllReduce.
                # Last pair split into single-m-tile groups so the final
                # (exposed) AllReduce is half the size with half the tail.
                GROUPS = [[0, 1], [2, 3], [4, 5], [6], [7]]
                for gi, grp in enumerate(GROUPS):
                    pair = grp[0] // 2
                    pg = {m: ps1.tile([128, B], F32, tag="psmm1",
                                      name=f"ps1_{m}") for m in grp}
                    for kt in range(NKT1):
                        wgt = wg1h[pair][kt // (NKT1 // 4)]
                        ko = kt % (NKT1 // 4)
                        xsrc = xtt[kt // (NKT1 // 8)]
                        xo = kt % (NKT1 // 8)
                        for m in grp:
                            jc = m % 2
                            nc.tensor.matmul(
                                pg[m][:],
                                wgt[:, ko * 256 + jc * 128:ko * 256 + (jc + 1) * 128],
                                xsrc[:, xo * B:(xo + 1) * B],
                                start=(kt == 0), stop=(kt == NKT1 - 1))
                    for m in grp:
                        cp = castp.tile([128, B], BF16, tag="cast", name=f"cp_{m}")
                        nc.scalar.activation(cp[:], pg[m][:], AF.Identity,
                                             bias=bg1c[:, m:m + 1], scale=1.0)
                        nc.scalar.dma_start(arin[m], cp[:])
                    if gi < 2:
                        for h in range(4):
                            nc.sync.dma_start(wg1h[gi + 2][h][:], wg1_d[gi + 2, h])
                    if stage < 2:
                        continue
                    lo, hi = grp[0], grp[-1] + 1
                    if sim:
                        nc.gpsimd.dma_start(arout[lo:hi], arin[lo:hi])
                    else:
                        nc.gpsimd.collective_compute(
                            "AllReduce", OP.add,
                            replica_groups=[list(range(NCORES))],
                            ins=[arin[lo:hi]], outs=[arout[lo:hi]],
                        )
                    for m in grp:
                        nc.sync.dma_start(h1m[m][:], arout[m])
                        nc.scalar.activation(e1m[m][:], h1m[m][:],
                                             AF.Exp, bias=neg1[:])
                        nc.vector.scalar_tensor_tensor(a1m[m][:], e1m[m][:], 1.0,
                                                       h1m[m][:], OP.min, OP.max)

                if stage >= 2:
                    # all MM2 partials AFTER the MM1 stream: groups 0-2 are
                    # long ready (they fill the PE gap while AR of group 3
                    # lands); only group 3's partials wait on its AllReduce.
                    for kt in range(8):
                        for mt in range(4):
                            nc.tensor.matmul(
                                ps2[mt][:],
                                wg2sb[:, kt * PD + mt * 128:kt * PD + (mt + 1) * 128],
                                a1m[kt][:],
                                start=(kt == 0), stop=(kt == 7))
                        if kt == 0:
                            for mt in range(4):
                                nc.tensor.matmul(ps2[mt][:],
                                                 b2r[:, mt * 128:(mt + 1) * 128],
                                                 ones2[:], start=False,
                                                 stop=False)

                if stage >= 2:
                    for mt in range(4):
                        eh = ep.tile([128, B], BF16, tag="escr", name=f"eh_{mt}")
                        nc.scalar.activation(eh[:], ps2[mt][:], AF.Exp, bias=neg1[:])
                        nc.vector.scalar_tensor_tensor(htm[mt][:], eh[:], 1.0,
                                                       ps2[mt][:], OP.min, OP.max)

            if stage >= 3:
                # ---- stage 2: per-dim nets, joint + marg ----
                rsumJ = wk.tile([128, DL], F32)
                ets = wk.tile([1, DL], F32)

                with (
                    tc.tile_pool(name="psL1", bufs=4, space="PSUM") as psL1,
                    tc.tile_pool(name="psL2", bufs=2, space="PSUM") as psL2,
                    tc.tile_pool(name="psm5", bufs=2, space="PSUM") as psm5,
                ):
                    def layer1(zrows, d, nm):
                        pre = psL1.tile([128, B], F32, tag="psL1", name=f"pL1_{nm}")
                        for kt in range(4):
                            nc.tensor.matmul(
                                pre[:],
                                wht[:, kt * DL * H + d * H:kt * DL * H + (d + 1) * H],
                                htm[kt][:],
                                start=(kt == 0), stop=False)
                        nc.tensor.matmul(pre[:], l1b[:, d * H:(d + 1) * H],
                                         zrows[:, d * B:(d + 1) * B],
                                         start=False, stop=True)
                        e = ep.tile([128, B], BF16, tag="escr", name=f"e_{nm}")
                        nc.scalar.activation(e[:], pre[:], AF.Exp, bias=neg1[:])
                        a = app.tile([128, B], BF16, tag="act", name=f"a_{nm}")
                        nc.vector.scalar_tensor_tensor(a[:], e[:], 1.0, pre[:],
                                                       OP.min, OP.max)
                        return a

                    def layer2(a, d, accum, nm):
                        pre = psL2.tile([128, B], F32, tag="psL2", name=f"pL2_{nm}")
                        nc.tensor.matmul(pre[:], w2t[:, d * H:(d + 1) * H], a[:],
                                         start=True, stop=False)
                        nc.tensor.matmul(pre[:], l2b[:, d * H:(d + 1) * H],
                                         ones2[:], start=False, stop=True)
                        e = ep.tile([128, B], BF16, tag="escr", name=f"e2_{nm}")
                        nc.scalar.activation(e[:], pre[:], AF.Exp, bias=neg1[:])
                        a2 = app.tile([128, B], BF16, tag="act", name=f"a2_{nm}")
                        kw = {"accum_out": rsumJ[:, d:d + 1]} if accum else {}
                        nc.vector.scalar_tensor_tensor(a2[:], e[:], 1.0, pre[:],
                                                       OP.min, OP.max, **kw)
                        return a2

                    # 2-stage software pipeline so PE's FIFO never waits on
                    # an elu chain: L1(d) || L2(d-1) || score(d-2)
                    aL1, aL2 = {}, {}
                    for d in range(DL + 2):
                        if d < DL:
                            aL1[d] = (layer1(zj, d, f"J{d}"),
                                      layer1(zm, d, f"M{d}"))
                        if 1 <= d <= DL:
                            aJ, aM = aL1.pop(d - 1)
                            aL2[d - 1] = (layer2(aJ, d - 1, True, f"J{d-1}"),
                                          layer2(aM, d - 1, False, f"M{d-1}"))
                        if d >= 2:
                            _, aM2 = aL2.pop(d - 2)
                            mrow = psm5.tile([128, B], F32, tag="psm5",
                                             name=f"m5_{d-2}")
                            nc.tensor.matmul(mrow[0:1, :], w3t[:, d - 2:d - 1],
                                             aM2[:], start=True, stop=True)
                            mear = mep.tile([1, B], F32, tag="mescr",
                                            name=f"me_{d-2}")
                            nc.scalar.activation(mear[:], mrow[0:1, :], AF.Exp,
                                                 bias=cvec[0:1, d - 2:d - 1],
                                                 accum_out=ets[0:1, d - 2:d - 1])

                # joint per-partition dot: jpp[k] = sum_d rsumJ[k,d]*W3[k,d]
                jsc = wk.tile([128, DL], F32)
                jpp = wk.tile([128, 1], F32)
                nc.vector.scalar_tensor_tensor(jsc[:], rsumJ[:], 1.0, w3t[:],
                                               OP.mult, OP.mult, accum_out=jpp[:])
                nc.sync.dma_start(out_d[0:DL], ets[0:1, :])
                nc.sync.dma_start(out_d[DL:DL + 128], jpp[:, 0:1])
    nc.compile()
    return nc


def _hilo(v):
    hi = v.astype(BF)
    lo = (v - hi.astype(np.float32)).astype(BF)
    return hi, lo


def _prep(x, z, perm, Wg1, bg1, Wg2, bg2, Wh, Wz, b1, W2, b2, W3, b3):
    """Build per-core input maps + host-side constants."""
    invperm = np.argsort(perm)
    zinv = z[invperm]                       # [B, D]
    bg2a = bg2 - Wg2.sum(axis=0)            # a1 shift correction
    b1a = b1 - Wh.sum(axis=1)               # [D, H] h~ shift correction
    b2a = b2 - W2.sum(axis=1)               # [D, H] a1 shift correction
    cj = b3 - W3.sum(axis=1)                # [D] a2 shift correction

    b2r_hi, b2r_lo = _hilo(1.0 + bg2a)
    ones2 = np.ones((2, B), BF)

    in_maps = []
    for c in range(NCORES):
        ksl = slice(c * KSH, (c + 1) * KSH)
        dsl = slice(c * DL, (c + 1) * DL)
        # xt[q][p, ko*B+b] = x[b, c*KSH + (q*8+ko)*128 + p]
        xt = np.ascontiguousarray(
            x[:, ksl].T.reshape(NKT1, 128, B).transpose(1, 0, 2)
            .reshape(128, 8, (NKT1 // 8) * B).transpose(1, 0, 2)).astype(BF)
        # wg1[g, h][p, ko*256+u] = Wg1_c[(h*16+ko)*128+p, g*256+u]
        wg1 = np.ascontiguousarray(
            Wg1[ksl].reshape(NKT1, 128, NG, 256).transpose(2, 1, 0, 3)
            .reshape(NG, 128, 4, (NKT1 // 4) * 256).transpose(0, 2, 1, 3)).astype(BF)
        bg1c = ((bg1 + 1.0) / 8.0).astype(np.float32).reshape(8, 128).T.copy()
        # wg2[p, kt*PD+m] = Wg2[kt*128+p, m]
        wg2 = np.ascontiguousarray(
            Wg2.reshape(8, 128, PD).transpose(1, 0, 2).reshape(128, 8 * PD)
        ).astype(BF)
        # wh[p, kt*DL*H + d*H+h] = Wh[dg, kt*128+p, h]
        wh = np.ascontiguousarray(
            Wh[dsl].transpose(1, 0, 2).reshape(4, 128, DL * H)
            .transpose(1, 0, 2).reshape(128, 4 * DL * H)).astype(BF)
        l1 = np.zeros((3, DL * H), np.float32)
        l1[0] = Wz[dsl].reshape(-1)
        v1hi, v1lo = _hilo((1.0 + b1a[dsl]).reshape(-1))
        l1b = np.stack([l1[0].astype(BF), v1hi, v1lo]).astype(BF)
        zjr = np.zeros((3, DL * B), np.float32)
        zmr = np.zeros((3, DL * B), np.float32)
        zjr[0] = z[:, dsl].T.reshape(-1)
        zmr[0] = zinv[:, dsl].T.reshape(-1)
        zjr[1:] = 1.0
        zmr[1:] = 1.0
        w2 = np.ascontiguousarray(
            W2[dsl].transpose(1, 0, 2).reshape(H, DL * H)
        ).astype(BF)
        v2hi, v2lo = _hilo((1.0 + b2a[dsl]).reshape(-1))
        l2b = np.stack([v2hi, v2lo])
        w3 = np.ascontiguousarray(W3[dsl].T).astype(BF)   # [H, DL]
        cvec = cj[dsl].reshape(1, DL).astype(np.float32)
        in_maps.append({
            "xt": xt, "wg1": wg1, "bg1c": bg1c, "wg2": wg2,
            "b2r": np.stack([b2r_hi, b2r_lo]),
            "wh": wh, "l1b": l1b,
            "zj": zjr.astype(BF), "zm": zmr.astype(BF),
            "w2": w2, "l2b": l2b, "ones2": ones2,
            "w3": w3, "cvec": cvec,
        })
    return in_maps, cj


def _combine(results, cj):
    """Host-side final reduction: 64 logs + means."""
    joint_sum = 0.0
    log_sum = 0.0
    for c in range(NCORES):
        o = results[c]["out"].astype(np.float64)
        joint_sum += o[DL:DL + 128].sum() / B
        ets = o[:DL]
        log_sum += np.log(ets / B).sum()
    joint_sum += cj.astype(np.float64).sum()
    mi_sum = joint_sum - log_sum
    return np.float32(-mi_sum / D)


def kernel(x, z, perm, Wg1, bg1, Wg2, bg2, Wh, Wz, b1, W2, b2, W3, b3):
    args = (x, z, perm, Wg1, bg1, Wg2, bg2, Wh, Wz, b1, W2, b2, W3, b3)
    args = tuple(np.asarray(a) for a in args)
    in_maps, cj = _prep(*args)
    if "nc" not in _cache:
        _cache["nc"] = _build()
    r = run_bass_kernel_spmd(_cache["nc"], in_maps, list(range(NCORES)))
    return _combine(r.results, cj)


# revision 26
# speedup vs baseline: 1.0028x; 1.0028x over previous
"""DimensionWiseMINE on 8 Trainium2 NeuronCores.

Sharding: stage-1 gene projector x@Wg1 is contraction-sharded (XD/8 per core)
with a chunked bf16 AllReduce of the pre-activation overlapping the matmul;
everything downstream of h is expert-parallel over the D=64 per-dim nets
(8 dims per core). The batch permutation is applied on the host to z (the
batch sum is permutation invariant), so no on-device gather is needed.

Layouts are feature-major ("transposed": [feature_partition, batch_free])
throughout, so no on-device transposes are needed.

ELU is computed as  elu(y)+1 = max(y+1, min(exp(y), 1))  -- one ACT op
(exp with bias -1 reading y+1) plus one fused DVE scalar_tensor_tensor op.
The +1 shifts are absorbed into the next layer's bias via weight row/col
sums, and all biases ride the matmuls as extra contraction rows (hi/lo bf16
split for accuracy) or ACT bias slots.
"""

import numpy as np
import ml_dtypes

import concourse.bass as bass
import concourse.bacc as bacc
import concourse.tile as tile
from concourse import mybir
from concourse.bass_utils import run_bass_kernel_spmd

NCORES = 8
B, XD, PD, D, H, G1 = 512, 32768, 512, 64, 128, 1024
KSH = XD // NCORES   # 4096 contraction slice per core
DL = D // NCORES     # 8 local dims per core
NKT1 = KSH // 128    # 32 k-tiles for MM1
NG = 4               # MM1 m-groups (each 2 m-tiles of g1) / AllReduce chunks
F32 = mybir.dt.float32
BF16 = mybir.dt.bfloat16
BF = ml_dtypes.bfloat16
AF = mybir.ActivationFunctionType
OP = mybir.AluOpType

_cache = {}


def _build(sim=False, stage=3):
    nc = bacc.Bacc("TRN2", target_bir_lowering=False, debug=False,
                   num_devices=1 if sim else NCORES)

    xt_d = nc.declare_dram_parameter("xt", [8, 128, (NKT1 // 8) * B], BF16, isOutput=False)
    wg1_d = nc.declare_dram_parameter("wg1", [NG, 4, 128, (NKT1 // 4) * 256], BF16, isOutput=False)
    bg1c_d = nc.declare_dram_parameter("bg1c", [128, 8], F32, isOutput=False)
    wg2_d = nc.declare_dram_parameter("wg2", [128, 8 * PD], BF16, isOutput=False)
    b2r_d = nc.declare_dram_parameter("b2r", [2, PD], BF16, isOutput=False)
    wh_d = nc.declare_dram_parameter("wh", [128, 4 * DL * H], BF16, isOutput=False)
    l1b_d = nc.declare_dram_parameter("l1b", [3, DL * H], BF16, isOutput=False)
    zj_d = nc.declare_dram_parameter("zj", [3, DL * B], BF16, isOutput=False)
    zm_d = nc.declare_dram_parameter("zm", [3, DL * B], BF16, isOutput=False)
    w2_d = nc.declare_dram_parameter("w2", [128, DL * H], BF16, isOutput=False)
    l2b_d = nc.declare_dram_parameter("l2b", [2, DL * H], BF16, isOutput=False)
    ones_d = nc.declare_dram_parameter("ones2", [2, B], BF16, isOutput=False)
    w3_d = nc.declare_dram_parameter("w3", [128, DL], BF16, isOutput=False)
    cvec_d = nc.declare_dram_parameter("cvec", [1, DL], F32, isOutput=False)
    out_d = nc.declare_dram_parameter("out", [144], F32, isOutput=True)

    arin = nc.dram_tensor("arin", [8, 128, B], BF16)
    arout = nc.dram_tensor("arout", [8, 128, B], BF16, addr_space="Shared")

    with tile.TileContext(nc) as tc:
        with (
            tc.tile_pool(name="wg1p", bufs=2) as wg1p,
            tc.tile_pool(name="castp", bufs=4) as castp,
            tc.tile_pool(name="consts", bufs=1) as cst,
            tc.tile_pool(name="work", bufs=1) as wk,
            tc.tile_pool(name="ep", bufs=6) as ep,
            tc.tile_pool(name="ap_", bufs=6) as app,
            tc.tile_pool(name="mep", bufs=4) as mep,
        ):
            # ---- startup DMAs in priority order: first MM1 operands,
            # then the rest of x / Wg1 group 0, then constants ----
            XC = (NKT1 // 8) * B
            xtt = [cst.tile([128, XC], BF16, name=f"xts_{i}") for i in range(8)]
            WQ = (NKT1 // 4) * 256
            wg1h = [[wg1p.tile([128, WQ], BF16, tag=f"wg1h{h}",
                               name=f"wg_{g}_{h}", bufs=3) for h in range(4)]
                    for g in range(NG)]
            # interleave so the first matmuls' operands land first
            nc.sync.dma_start(xtt[0][:], xt_d[0])
            nc.sync.dma_start(wg1h[0][0][:], wg1_d[0, 0])
            nc.sync.dma_start(xtt[1][:], xt_d[1])
            nc.sync.dma_start(wg1h[0][1][:], wg1_d[0, 1])
            nc.sync.dma_start(xtt[2][:], xt_d[2])
            nc.sync.dma_start(wg1h[0][2][:], wg1_d[0, 2])
            nc.sync.dma_start(xtt[3][:], xt_d[3])
            nc.sync.dma_start(wg1h[0][3][:], wg1_d[0, 3])
            for i in range(4, 8):
                nc.sync.dma_start(xtt[i][:], xt_d[i])
            for h in range(4):
                nc.sync.dma_start(wg1h[1][h][:], wg1_d[1, h])
            # ---- constants in (lower priority) ----
            bg1c = cst.tile([128, 8], F32)
            nc.scalar.dma_start(bg1c[:], bg1c_d[:])
            wg2sb = cst.tile([128, 8 * PD], BF16)
            nc.scalar.dma_start(wg2sb[:], wg2_d[:])
            b2r = cst.tile([2, PD], BF16)
            nc.scalar.dma_start(b2r[:], b2r_d[:])
            wht = cst.tile([128, 4 * DL * H], BF16)
            nc.scalar.dma_start(wht[:], wh_d[:])
            l1b = cst.tile([3, DL * H], BF16)
            nc.scalar.dma_start(l1b[:], l1b_d[:])
            zj = cst.tile([3, DL * B], BF16)
            nc.scalar.dma_start(zj[:], zj_d[:])
            zm = cst.tile([3, DL * B], BF16)
            nc.scalar.dma_start(zm[:], zm_d[:])
            w2t = cst.tile([128, DL * H], BF16)
            nc.scalar.dma_start(w2t[:], w2_d[:])
            l2b = cst.tile([2, DL * H], BF16)
            nc.scalar.dma_start(l2b[:], l2b_d[:])
            ones2 = cst.tile([2, B], BF16)
            nc.scalar.dma_start(ones2[:], ones_d[:])
            w3t = cst.tile([128, DL], BF16)
            nc.scalar.dma_start(w3t[:], w3_d[:])
            cvec = cst.tile([1, DL], F32)
            nc.scalar.dma_start(cvec[:], cvec_d[:])

            # warm the exp table set early so the ~2.7us load hides under MM1
            wz0 = cst.tile([128, 1], F32)
            nc.vector.memset(wz0[:], 0.0)
            wz1 = cst.tile([128, 1], F32)
            nc.scalar.activation(wz1[:], wz0[:], AF.Exp)
            neg1 = cst.tile([128, 1], F32)
            nc.vector.memset(neg1[:], -1.0)


            h1m = [wk.tile([128, B], BF16, name=f"h1m_{m}") for m in range(8)]
            e1m = [wk.tile([128, B], BF16, name=f"e1m_{m}") for m in range(8)]
            a1m = [wk.tile([128, B], BF16, name=f"a1m_{m}") for m in range(8)]
            htm = [wk.tile([128, B], BF16, name=f"htm_{mt}") for mt in range(4)]

            with (
                tc.tile_pool(name="ps1", bufs=4, space="PSUM") as ps1,
                tc.tile_pool(name="ps2p", bufs=4, space="PSUM") as ps2p,
            ):
                ps2 = [ps2p.tile([128, B], F32, tag="psmm2", name=f"psmm2_{i}")
                       for i in range(4)]
                # ---- MM1 k-contiguous per m-group + chunked AllReduce.
                # Last pair split into single-m-tile groups so the final
                # (exposed) AllReduce is half the size with half the tail.
                GROUPS = [[0, 1], [2, 3], [4, 5], [6], [7]]
                for gi, grp in enumerate(GROUPS):
                    pair = grp[0] // 2
                    pg = {m: ps1.tile([128, B], F32, tag="psmm1",
                                      name=f"ps1_{m}") for m in grp}
                    for kt in range(NKT1):
                        wgt = wg1h[pair][kt // (NKT1 // 4)]
                        ko = kt % (NKT1 // 4)
                        xsrc = xtt[kt // (NKT1 // 8)]
                        xo = kt % (NKT1 // 8)
                        for m in grp:
                            jc = m % 2
                            nc.tensor.matmul(
                                pg[m][:],
                                wgt[:, ko * 256 + jc * 128:ko * 256 + (jc + 1) * 128],
                                xsrc[:, xo * B:(xo + 1) * B],
                                start=(kt == 0), stop=(kt == NKT1 - 1))
                    for m in grp:
                        cp = castp.tile([128, B], BF16, tag="cast", name=f"cp_{m}")
                        nc.scalar.activation(cp[:], pg[m][:], AF.Identity,
                                             bias=bg1c[:, m:m + 1], scale=1.0)
                        nc.scalar.dma_start(arin[m], cp[:])
                    if gi < 2:
                        for h in range(4):
                            nc.sync.dma_start(wg1h[gi + 2][h][:], wg1_d[gi + 2, h])
                    if stage < 2:
                        continue
                    lo, hi = grp[0], grp[-1] + 1
                    if sim:
                        nc.gpsimd.dma_start(arout[lo:hi], arin[lo:hi])
                    else:
                        nc.gpsimd.collective_compute(
                            "AllReduce", OP.add,
                            replica_groups=[list(range(NCORES))],
                            ins=[arin[lo:hi]], outs=[arout[lo:hi]],
                        )
                    for m in grp:
                        nc.sync.dma_start(h1m[m][:], arout[m])
                        nc.scalar.activation(e1m[m][:], h1m[m][:],
                                             AF.Exp, bias=neg1[:])
                        nc.vector.scalar_tensor_tensor(a1m[m][:], e1m[m][:], 1.0,
                                                       h1m[m][:], OP.min, OP.max)

                if stage >= 2:
                    # all MM2 partials AFTER the MM1 stream: groups 0-2 are
                    # long ready (they fill the PE gap while AR of group 3
                    # lands); only group 3's partials wait on its AllReduce.
                    for kt in range(8):
                        for mt in range(4):
                            nc.tensor.matmul(
                                ps2[mt][:],
                                wg2sb[:, kt * PD + mt * 128:kt * PD + (mt + 1) * 128],
                                a1m[kt][:],
                                start=(kt == 0), stop=(kt == 7))
                        if kt == 0:
                            for mt in range(4):
                                nc.tensor.matmul(ps2[mt][:],
                                                 b2r[:, mt * 128:(mt + 1) * 128],
                                                 ones2[:], start=False,
                                                 stop=False)

                if stage >= 2:
                    for mt in range(4):
                        eh = ep.tile([128, B], BF16, tag="escr", name=f"eh_{mt}")
                        nc.scalar.activation(eh[:], ps2[mt][:], AF.Exp, bias=neg1[:])
                        nc.vector.scalar_tensor_tensor(htm[mt][:], eh[:], 1.0,
                                                       ps2[mt][:], OP.min, OP.max)

            if stage >= 3:
                # ---- stage 2: per-dim nets, joint + marg ----
                rsumJ = wk.tile([128, DL], F32)
                ets = wk.tile([1, DL], F32)

                with (
                    tc.tile_pool(name="psL1", bufs=4, space="PSUM") as psL1,
                    tc.tile_pool(name="psL2", bufs=3, space="PSUM") as psL2,
                    tc.tile_pool(name="psm5", bufs=1, space="PSUM") as psm5,
                ):
                    def layer1(zrows, d, nm):
                        pre = psL1.tile([128, B], F32, tag="psL1", name=f"pL1_{nm}")
                        for kt in range(4):
                            nc.tensor.matmul(
                                pre[:],
                                wht[:, kt * DL * H + d * H:kt * DL * H + (d + 1) * H],
                                htm[kt][:],
                                start=(kt == 0), stop=False)
                        nc.tensor.matmul(pre[:], l1b[:, d * H:(d + 1) * H],
                                         zrows[:, d * B:(d + 1) * B],
                                         start=False, stop=True)
                        e = ep.tile([128, B], BF16, tag="escr", name=f"e_{nm}")
                        nc.scalar.activation(e[:], pre[:], AF.Exp, bias=neg1[:])
                        a = app.tile([128, B], BF16, tag="act", name=f"a_{nm}")
                        nc.vector.scalar_tensor_tensor(a[:], e[:], 1.0, pre[:],
                                                       OP.min, OP.max)
                        return a

                    def layer2(a, d, accum, nm):
                        pre = psL2.tile([128, B], F32, tag="psL2", name=f"pL2_{nm}")
                        nc.tensor.matmul(pre[:], w2t[:, d * H:(d + 1) * H], a[:],
                                         start=True, stop=False)
                        nc.tensor.matmul(pre[:], l2b[:, d * H:(d + 1) * H],
                                         ones2[:], start=False, stop=True)
                        e = ep.tile([128, B], BF16, tag="escr", name=f"e2_{nm}")
                        nc.scalar.activation(e[:], pre[:], AF.Exp, bias=neg1[:])
                        a2 = app.tile([128, B], BF16, tag="act", name=f"a2_{nm}")
                        kw = {"accum_out": rsumJ[:, d:d + 1]} if accum else {}
                        nc.vector.scalar_tensor_tensor(a2[:], e[:], 1.0, pre[:],
                                                       OP.min, OP.max, **kw)
                        return a2

                    # 2-stage software pipeline so PE's FIFO never waits on
                    # an elu chain: L1(d) || L2(d-1) || score(d-2)
                    aL1, aL2 = {}, {}
                    for d in range(DL + 2):
                        if d < DL:
                            aL1[d] = (layer1(zj, d, f"J{d}"),
                                      layer1(zm, d, f"M{d}"))
                        if 1 <= d <= DL:
                            aJ, aM = aL1.pop(d - 1)
                            aL2[d - 1] = (layer2(aJ, d - 1, True, f"J{d-1}"),
                                          layer2(aM, d - 1, False, f"M{d-1}"))
                        if d >= 2:
                            _, aM2 = aL2.pop(d - 2)
                            mrow = psm5.tile([128, B], F32, tag="psm5",
                                             name=f"m5_{d-2}")
                            nc.tensor.matmul(mrow[0:1, :], w3t[:, d - 2:d - 1],
                                             aM2[:], start=True, stop=True)
                            mear = mep.tile([1, B], F32, tag="mescr",
                                            name=f"me_{d-2}")
                            nc.scalar.activation(mear[:], mrow[0:1, :], AF.Exp,
                                                 bias=cvec[0:1, d - 2:d - 1],
                                                 accum_out=ets[0:1, d - 2:d - 1])

                # joint per-partition dot: jpp[k] = sum_d rsumJ[k,d]*W3[k,d]
                jsc = wk.tile([128, DL], F32)
                jpp = wk.tile([128, 1], F32)
                nc.vector.scalar_tensor_tensor(jsc[:], rsumJ[:], 1.0, w3t[:],
                                               OP.mult, OP.mult, accum_out=jpp[:])
                nc.sync.dma_start(out_d[0:DL], ets[0:1, :])
                nc.sync.dma_start(out_d[DL:DL + 128], jpp[:, 0:1])
    nc.compile()
    return nc


def _hilo(v):
    hi = v.astype(BF)
    lo = (v - hi.astype(np.float32)).astype(BF)
    return hi, lo


def _prep(x, z, perm, Wg1, bg1, Wg2, bg2, Wh, Wz, b1, W2, b2, W3, b3):
    """Build per-core input maps + host-side constants."""
    invperm = np.argsort(perm)
    zinv = z[invperm]                       # [B, D]
    bg2a = bg2 - Wg2.sum(axis=0)            # a1 shift correction
    b1a = b1 - Wh.sum(axis=1)               # [D, H] h~ shift correction
    b2a = b2 - W2.sum(axis=1)               # [D, H] a1 shift correction
    cj = b3 - W3.sum(axis=1)                # [D] a2 shift correction

    b2r_hi, b2r_lo = _hilo(1.0 + bg2a)
    ones2 = np.ones((2, B), BF)

    in_maps = []
    for c in range(NCORES):
        ksl = slice(c * KSH, (c + 1) * KSH)
        dsl = slice(c * DL, (c + 1) * DL)
        # xt[q][p, ko*B+b] = x[b, c*KSH + (q*8+ko)*128 + p]
        xt = np.ascontiguousarray(
            x[:, ksl].T.reshape(NKT1, 128, B).transpose(1, 0, 2)
            .reshape(128, 8, (NKT1 // 8) * B).transpose(1, 0, 2)).astype(BF)
        # wg1[g, h][p, ko*256+u] = Wg1_c[(h*16+ko)*128+p, g*256+u]
        wg1 = np.ascontiguousarray(
            Wg1[ksl].reshape(NKT1, 128, NG, 256).transpose(2, 1, 0, 3)
            .reshape(NG, 128, 4, (NKT1 // 4) * 256).transpose(0, 2, 1, 3)).astype(BF)
        bg1c = ((bg1 + 1.0) / 8.0).astype(np.float32).reshape(8, 128).T.copy()
        # wg2[p, kt*PD+m] = Wg2[kt*128+p, m]
        wg2 = np.ascontiguousarray(
            Wg2.reshape(8, 128, PD).transpose(1, 0, 2).reshape(128, 8 * PD)
        ).astype(BF)
        # wh[p, kt*DL*H + d*H+h] = Wh[dg, kt*128+p, h]
        wh = np.ascontiguousarray(
            Wh[dsl].transpose(1, 0, 2).reshape(4, 128, DL * H)
            .transpose(1, 0, 2).reshape(128, 4 * DL * H)).astype(BF)
        l1 = np.zeros((3, DL * H), np.float32)
        l1[0] = Wz[dsl].reshape(-1)
        v1hi, v1lo = _hilo((1.0 + b1a[dsl]).reshape(-1))
        l1b = np.stack([l1[0].astype(BF), v1hi, v1lo]).astype(BF)
        zjr = np.zeros((3, DL * B), np.float32)
        zmr = np.zeros((3, DL * B), np.float32)
        zjr[0] = z[:, dsl].T.reshape(-1)
        zmr[0] = zinv[:, dsl].T.reshape(-1)
        zjr[1:] = 1.0
        zmr[1:] = 1.0
        w2 = np.ascontiguousarray(
            W2[dsl].transpose(1, 0, 2).reshape(H, DL * H)
        ).astype(BF)
        v2hi, v2lo = _hilo((1.0 + b2a[dsl]).reshape(-1))
        l2b = np.stack([v2hi, v2lo])
        w3 = np.ascontiguousarray(W3[dsl].T).astype(BF)   # [H, DL]
        cvec = cj[dsl].reshape(1, DL).astype(np.float32)
        in_maps.append({
            "xt": xt, "wg1": wg1, "bg1c": bg1c, "wg2": wg2,
            "b2r": np.stack([b2r_hi, b2r_lo]),
            "wh": wh, "l1b": l1b,
            "zj": zjr.astype(BF), "zm": zmr.astype(BF),
            "w2": w2, "l2b": l2b, "ones2": ones2,
            "w3": w3, "cvec": cvec,
        })
    return in_maps, cj


def _combine(results, cj):
    """Host-side final reduction: 64 logs + means."""
    joint_sum = 0.0
    log_sum = 0.0
    for c in range(NCORES):
        o = results[c]["out"].astype(np.float64)
        joint_sum += o[DL:DL + 128].sum() / B
        ets = o[:DL]
        log_sum += np.log(ets / B).sum()
    joint_sum += cj.astype(np.float64).sum()
    mi_sum = joint_sum - log_sum
    return np.float32(-mi_sum / D)


def kernel(x, z, perm, Wg1, bg1, Wg2, bg2, Wh, Wz, b1, W2, b2, W3, b3):
    args = (x, z, perm, Wg1, bg1, Wg2, bg2, Wh, Wz, b1, W2, b2, W3, b3)
    args = tuple(np.asarray(a) for a in args)
    in_maps, cj = _prep(*args)
    if "nc" not in _cache:
        _cache["nc"] = _build()
    r = run_bass_kernel_spmd(_cache["nc"], in_maps, list(range(NCORES)))
    return _combine(r.results, cj)
